# revision 1
# baseline (speedup 1.0000x reference)
"""Trainium2 Bass kernel for nn_Attention_36404142801494.

Fused causal self-attention (q=k=v=Wq(x)) + output projection, sharded over
8 NeuronCores: data-parallel on batch (B=2 -> 2 groups of 4 cores), tensor-
parallel on heads (8 heads -> 2 heads/core) with a column-split Wq and a
row-split Wo. Each core returns a partial [S, HID] output; the host sums the
4 partials per batch and adds the Wo bias while unsharding.

Layout strategy on device (per core):
  - qT [d, s] layout (d on partitions) so QK^T tiles come out as
    scoresT [t_keys=128, s_queries=512] and feed the AV matmul directly.
  - scores for one (head, 512-query block) are built 3 key-chunks at a time
    into a 3-bank PSUM group [128, 1536] (double buffered), exponentiated by
    one ACT pass per group (scale=1/8 folded in, bf16 out), causal-masked
    per diagonal segment via precomputed 0/1 mask multiplies.
  - V tiles [t, d] come from PE transposes of qT; 32 extra "ones" columns
    make the AV matmul emit softmax denominators (rows 64..95 of the AV
    accumulator) for free.
  - Normalization: denominator row -> DRAM bounce -> partition-broadcast
    DMA -> reciprocal -> one multiply while copying the AV result out.
  - QK / projections run as float32r (full-rate fp32 PE mode, ~1.6e-4 rel
    err); exp'd scores and V run in bf16 (post-softmax values, error is
    averaged out by the AV reduction).

Everything is hardcoded for B=2, S=2048, HID=512, NH=8, HD=64.
"""

import sys

sys.path.insert(0, "/opt/trn_rl_repo")

import numpy as np

import concourse.bass as bass
import concourse.bacc as bacc
import concourse.tile as tile
import concourse.mybir as mybir
from concourse.bass_utils import run_bass_kernel_spmd
from concourse.masks import make_identity

f32 = mybir.dt.float32
f32r = mybir.dt.float32r
bf16 = mybir.dt.bfloat16
EXPT_DT = bf16  # dtype of exp'd scores + V (AV matmul operands)

B, S, HID = 2, 2048, 512
NH, HD = 8, 64
N_CORES = 8
SB = 512           # query-block width (one PSUM bank of fp32)
NSB = S // SB      # 4 query blocks
NCH = S // 128     # 16 key chunks
GRP = 2            # key chunks per PSUM scores group (2 banks, double buffered)
SCALE = 1.0 / np.sqrt(HD)

Exp = mybir.ActivationFunctionType.Exp


def build_nc():
    """Build the (identical-on-every-core) Bass program."""
    nc = bacc.Bacc(None, target_bir_lowering=False)

    xT = nc.dram_tensor("xT", [HID, S], f32, kind="ExternalInput")
    WqT = nc.dram_tensor("WqT", [HID, 128], f32, kind="ExternalInput")
    Wqb = nc.dram_tensor("Wqb", [128, 1], f32, kind="ExternalInput")
    WoT = nc.dram_tensor("WoT", [128, HID], f32, kind="ExternalInput")
    dmask = nc.dram_tensor("dmask", [128, 2048], f32, kind="ExternalInput")
    out_part = nc.dram_tensor("out_part", [S, HID], f32, kind="ExternalOutput")

    with tile.TileContext(nc) as tc:
        with (
            tc.tile_pool(name="singles", bufs=1) as singles,
            tc.tile_pool(name="qtp", bufs=1) as qtp,
            tc.tile_pool(name="etp", bufs=12) as etp,
            tc.tile_pool(name="vp", bufs=1) as vp,
            tc.tile_pool(name="aop", bufs=1) as aop,
            tc.tile_pool(name="np_", bufs=4) as np_,
            tc.tile_pool(name="outp", bufs=4) as outp,
            tc.tile_pool(name="drp", bufs=2, space="DRAM") as drp,
        ):
            # ---- load constants / inputs (qproj-critical ones first) ----
            wq = singles.tile([128, 4, 128], f32r, tag="wq")
            for i in range(4):
                nc.sync.dma_start(
                    out=wq[:, i, :], in_=WqT[128 * i : 128 * (i + 1), :].bitcast(f32r)
                )
            wqb = singles.tile([128, 1], f32, tag="wqb")
            nc.sync.dma_start(out=wqb, in_=Wqb[:, :])

            # xT loaded per 512-column block so qproj can start early
            xs = [singles.tile([128, S], f32r, name=f"xt{i}", tag=f"xt{i}") for i in range(4)]
            for sb in range(NSB):
                s0 = sb * SB
                for i in range(4):
                    nc.sync.dma_start(
                        out=xs[i][:, s0 : s0 + SB],
                        in_=xT[128 * i : 128 * (i + 1), s0 : s0 + SB].bitcast(f32r),
                    )

            # non-critical loads go through the gpsimd DMA queue
            wo = singles.tile([64, 2 * HID], f32r, tag="wo")
            nc.gpsimd.dma_start(out=wo[:, 0:HID], in_=WoT[0:64, :].bitcast(f32r))
            nc.gpsimd.dma_start(out=wo[:, HID : 2 * HID], in_=WoT[64:128, :].bitcast(f32r))
            dm = singles.tile([128, 2048], EXPT_DT, tag="dm")
            dmf = singles.tile([128, 2048], f32, tag="dmf")
            nc.gpsimd.dma_start(out=dmf, in_=dmask[:, :])
            nc.gpsimd.tensor_copy(dm, dmf)

            # preload the exp ACT table while DMAs stream in
            preld = singles.tile([32, 32], f32, tag="preld")
            nc.vector.memset(preld, 0.0)
            nc.scalar.activation(out=preld, in_=preld, func=Exp, scale=1.0)

            identf = singles.tile([128, 64], f32, tag="identf")
            make_identity(nc, identf[0:64, :])
            nc.gpsimd.memset(identf[64:128, :], 0.0)
            nc.gpsimd.affine_select(
                out=identf[64:128, :], in_=identf[64:128, :],
                compare_op=mybir.AluOpType.not_equal,
                fill=1.0, base=0, pattern=[[-1, 64]], channel_multiplier=1,
            )
            ident = singles.tile([128, 64], f32r, tag="ident")
            nc.vector.tensor_copy(ident, identf)
            onesf = singles.tile([128, 32], f32, tag="onesf")
            nc.vector.memset(onesf, 1.0)

            qT = qtp.tile([128, S], f32r, tag="qT")
            v_sb = [vp.tile([128, NCH, 96], EXPT_DT, name=f"v{h}", tag=f"v{h}") for h in range(2)]
            ao = [aop.tile([64, S], f32r, name=f"ao{h}", tag=f"ao{h}") for h in range(2)]

            # ---- phase 1: q projection (qT = Wq @ x^T + b) and V prep ----


            # ---- main pipeline ----
            def vprep(vps, h, tq):
                hp = 64 * h
                vt = vps.tile([128, 4, 64], f32r, tag="ps1", name="vt")
                for j in range(4):
                    t0 = 128 * (4 * tq + j)
                    nc.tensor.transpose(
                        vt[:, j, :], qT[hp : hp + 64, t0 : t0 + 128],
                        ident[hp : hp + 64, :],
                    )
                nc.vector.tensor_copy(v_sb[h][:, 4 * tq : 4 * tq + 4, 0:64], vt)
                for j in range(4):
                    nc.gpsimd.tensor_copy(v_sb[h][:, 4 * tq + j, 64:96], onesf)

            def attention(qkps, avps, h, sb, tail=False):
                hp = 64 * h
                s0 = sb * SB
                nch = 4 * (sb + 1)
                groups = [
                    list(range(g0, min(g0 + GRP, nch))) for g0 in range(0, nch, GRP)
                ]
                av = avps.tile([96, SB], f32, tag="av", name="av")
                ets = []
                for chunks in groups:
                    gw = len(chunks)
                    qk = qkps.tile([128, GRP * SB], f32, tag="qk", name="qk")
                    for k, ci in enumerate(chunks):
                        t0 = 128 * ci
                        nc.tensor.matmul(
                            qk[:, SB * k : SB * (k + 1)],
                            lhsT=qT[hp : hp + 64, t0 : t0 + 128],
                            rhs=qT[hp : hp + 64, s0 : s0 + SB],
                            start=True, stop=True,
                        )
                    et = etp.tile([128, GRP * SB], EXPT_DT, tag="et", name="et")
                    nc.scalar.activation(
                        out=et[:, 0 : gw * SB], in_=qk[:, 0 : gw * SB],
                        func=Exp, scale=SCALE,
                    )
                    for k, ci in enumerate(chunks):
                        d = ci - 4 * sb
                        if d >= 0:
                            w = 128 * d + 128  # cols beyond are all-ones mask
                            nc.vector.tensor_mul(
                                et[:, SB * k : SB * k + w],
                                et[:, SB * k : SB * k + w],
                                dm[:, SB * d : SB * d + w],
                            )
                    ets.append((chunks, et))
                for chunks, et in ets:
                    for k, ci in enumerate(chunks):
                        nc.tensor.matmul(
                            av,
                            lhsT=v_sb[h][:, ci, :],
                            rhs=et[:, SB * k : SB * (k + 1)],
                            start=(ci == 0), stop=(ci == nch - 1),
                        )
                # normalization: denom rows 64..95 -> recip -> scale
                den = np_.tile([96, SB], f32, tag="den", name="den")
                if tail:
                    nc.scalar.copy(den, av)
                else:
                    nc.vector.tensor_copy(den, av)
                dr = drp.tile([1, SB], f32, name="dr")
                nc.sync.dma_start(out=dr[:, :], in_=den[64:65, :])
                drap = dr[:, :]
                bcast = bass.AP(
                    tensor=drap.tensor, offset=drap.offset,
                    ap=[[0, 64]] + list(drap.ap)[1:],
                )
                bcr = np_.tile([64, SB], f32, tag="bcr", name="bcr")
                nc.sync.dma_start(out=bcr, in_=bcast)
                bc = np_.tile([64, SB], f32, tag="bc", name="bc")
                nc.vector.reciprocal(bc, bcr)
                nc.vector.tensor_mul(ao[h][:, s0 : s0 + SB], den[0:64, :], bc)

            def wo_block(wops, sb):
                for sc in range(4 * sb, 4 * sb + 4):
                    c0 = 128 * sc
                    wp = wops.tile([128, HID], f32, tag="ps1", name="wp")
                    nc.tensor.matmul(
                        wp, lhsT=ao[0][:, c0 : c0 + 128], rhs=wo[:, 0:HID],
                        start=True, stop=False,
                    )
                    nc.tensor.matmul(
                        wp, lhsT=ao[1][:, c0 : c0 + 128], rhs=wo[:, HID : 2 * HID],
                        start=False, stop=True,
                    )
                    ob = outp.tile([128, HID], f32, tag="ob", name="ob")
                    nc.vector.tensor_copy(ob, wp)
                    nc.sync.dma_start(out=out_part[c0 : c0 + 128, :], in_=ob)

            with (
                tc.tile_pool(name="qpps", bufs=2, space="PSUM") as qpps,
                tc.tile_pool(name="qkps", bufs=2, space="PSUM") as qkps,
                tc.tile_pool(name="avps", bufs=2, space="PSUM") as avps,
            ):
                # qpps doubles as the vprep transpose pool and the Wo pool:
                # qproj ends as vprep starts, Wo comes later still.
                for sb in range(NSB):
                    s0 = sb * SB
                    qp = qpps.tile([128, SB], f32, tag="ps1", name="qp")
                    for i in range(4):
                        nc.tensor.matmul(
                            qp, lhsT=wq[:, i, :], rhs=xs[i][:, s0 : s0 + SB],
                            start=(i == 0), stop=(i == 3),
                        )
                    nc.vector.tensor_scalar_add(qT[:, s0 : s0 + SB], qp, wqb)
                    vprep(qpps, 0, sb)
                    vprep(qpps, 1, sb)
                    attention(qkps, avps, 0, sb)
                for sb in (3, 2, 1, 0):
                    attention(qkps, avps, 1, sb, tail=(sb <= 1))
                    wo_block(qpps, sb)

    nc.finalize()
    return nc


def _dmask():
    """[128, 2048] mask, segment d in {0..3}: keep (t + 128*d) <= j."""
    t = np.arange(128)[:, None]
    j = np.arange(512)[None, :]
    segs = [(t + 128 * k <= j).astype(np.float32) for k in range(4)]
    return np.concatenate(segs, axis=1)


_NC_CACHE = None


def _get_nc():
    global _NC_CACHE
    if _NC_CACHE is None:
        _NC_CACHE = build_nc()
    return _NC_CACHE


def make_in_maps(x, Wq_w, Wq_b, Wo_w):
    x = np.asarray(x, dtype=np.float32)
    Wq_w = np.asarray(Wq_w, dtype=np.float32)
    Wq_b = np.asarray(Wq_b, dtype=np.float32)
    Wo_w = np.asarray(Wo_w, dtype=np.float32)
    dmask = _dmask()
    in_maps = []
    for c in range(N_CORES):
        b, hp = divmod(c, 4)
        dq = slice(128 * hp, 128 * (hp + 1))
        in_maps.append({
            "xT": np.ascontiguousarray(x[b].T),
            "WqT": np.ascontiguousarray(Wq_w[dq, :].T),
            "Wqb": np.ascontiguousarray(Wq_b[dq].reshape(128, 1)),
            "WoT": np.ascontiguousarray(Wo_w[:, dq].T),
            "dmask": dmask,
        })
    return in_maps


def kernel(x, mask, Wq_w, Wq_b, Wo_w, Wo_b, **_):
    nc = _get_nc()
    in_maps = make_in_maps(x, Wq_w, Wq_b, Wo_w)
    res = run_bass_kernel_spmd(nc, in_maps, core_ids=list(range(N_CORES)))
    Wo_b = np.asarray(Wo_b, dtype=np.float32)
    out = np.empty((B, S, HID), dtype=np.float32)
    for b in range(B):
        acc = res.results[4 * b]["out_part"].astype(np.float32)
        for c in range(4 * b + 1, 4 * b + 4):
            acc = acc + res.results[c]["out_part"]
        out[b] = acc + Wo_b[None, :]
    return out



# revision 5
# speedup vs baseline: 1.2279x; 1.2279x over previous
"""Trainium2 Bass kernel for nn_Attention_36404142801494.

Fused causal self-attention (q=k=v=Wq(x)) + output projection, sharded over
8 NeuronCores: data-parallel on batch (B=2 -> 2 groups of 4 cores), tensor-
parallel on heads (8 heads -> 2 heads/core, dq = 128 hidden dims/core) with a
column-split Wq and a row-split Wo. Each core returns a partial [S, HID]
output (bf16); the host sums the 4 partials per batch and adds the Wo bias.

Per-core structure (keyed to the TimelineSim cost model, where a matmul costs
out_free_size x cycles_per_row and LDWEIGHTS is free):
  - qT [d=128, s] f32r via Wq matmuls (moving = x blocks, N=512/256).
  - QK emits scoresT tiles [k=128, q<=512] per key-chunk, causally trimmed;
    f32r keeps 1 cyc/row at N>=256 (diagonal chunks padded to N=256 min).
  - exp on ACT into bf16 et tiles; diagonal 128x128 blocks masked by a
    lower-triangular bf16 mask multiply on DVE (mask generated on device).
  - AV runs TRANSPOSED: stationary = et 128x128 block, moving = V chunk
    [k=128, 65] bf16 (col 64 = ones) -> av psum [q=128, 65], N=65/matmul.
    Col 64 accumulates the softmax denominator for free; normalization is a
    per-partition reciprocal + tensor_scalar multiply (no DRAM bounce).
  - ao [q, d] bf16 is PE-transposed (bf16 identity) to aoT [d, q]; both
    heads stack to [128, q] so Wo is one K=128 matmul per 128-query chunk
    (moving = WoT [128 d, 512 c] bf16, N=512).
  - Output partials DMA out as bf16 [2048, 512].

Everything is hardcoded for B=2, S=2048, HID=512, NH=8, HD=64.
"""

import sys

sys.path.insert(0, "/opt/trn_rl_repo")

import numpy as np

import concourse.bass as bass
import concourse.bacc as bacc
import concourse.tile as tile
import concourse.mybir as mybir
from concourse.bass_utils import run_bass_kernel_spmd

f32 = mybir.dt.float32
f32r = mybir.dt.float32r
bf16 = mybir.dt.bfloat16

B, S, HID = 2, 2048, 512
NH, HD = 8, 64
N_CORES = 8
SB = 512            # query-block width
NSB = S // SB       # 4 query blocks
SCALE = 1.0 / np.sqrt(HD)

Exp = mybir.ActivationFunctionType.Exp
ALU = mybir.AluOpType


def build_nc():
    nc = bacc.Bacc(None, target_bir_lowering=False)

    # host pre-arranged layouts (see make_in_maps):
    #   xB[p, i, s]  = x[b].T[128*i + p, s]
    #   WqB[p, i, c] = Wq_w[dq, :].T[128*i + p, c]
    #   WoT[d, c]    = Wo_w[:, dq].T[d, c]
    xB = nc.dram_tensor("xB", [128, 4, S], f32, kind="ExternalInput")
    WqB = nc.dram_tensor("WqB", [128, 4, 128], f32, kind="ExternalInput")
    Wqb = nc.dram_tensor("Wqb", [128, 1], f32, kind="ExternalInput")
    WoT = nc.dram_tensor("WoT", [128, HID], f32, kind="ExternalInput")
    out_part = nc.dram_tensor("out_part", [S, HID], bf16, kind="ExternalOutput")

    with tile.TileContext(nc) as tc:
        with (
            tc.tile_pool(name="singles", bufs=1) as singles,
            tc.tile_pool(name="etp", bufs=12) as etp,
            tc.tile_pool(name="aop", bufs=6) as aop,
            tc.tile_pool(name="recp", bufs=2) as recp,
            tc.tile_pool(name="aotsb", bufs=2) as aotsb,
            tc.tile_pool(name="obp", bufs=4) as obp,
            tc.tile_pool(name="qkp", bufs=2, space="PSUM") as qkp,
            tc.tile_pool(name="ppp", bufs=2, space="PSUM") as ppp,
            tc.tile_pool(name="avp", bufs=1, space="PSUM") as avp,
            tc.tile_pool(name="aotp", bufs=1, space="PSUM") as aotp,
        ):
            # ---------------- prologue: DMAs + constants ----------------
            wq = singles.tile([128, 4, 128], f32r, tag="wq")
            nc.sync.dma_start(out=wq, in_=WqB[:, :, :].bitcast(f32r))
            wqb = singles.tile([128, 1], f32, tag="wqb")
            nc.sync.dma_start(out=wqb, in_=Wqb[:, :])

            xs = singles.tile([128, 4, S], f32r, tag="xs")
            # qb0 in two halves so qproj(0) can start sooner
            nc.sync.dma_start(
                out=xs[:, :, 0:256], in_=xB[:, :, 0:256].bitcast(f32r)
            )
            nc.sync.dma_start(
                out=xs[:, :, 256:512], in_=xB[:, :, 256:512].bitcast(f32r)
            )
            nc.sync.dma_start(
                out=xs[:, :, 512:1024], in_=xB[:, :, 512:1024].bitcast(f32r)
            )
            woT_f = singles.tile([128, HID], f32, tag="woT_f")
            nc.sync.dma_start(out=woT_f, in_=WoT[:, :])
            nc.sync.dma_start(
                out=xs[:, :, 1024:1536], in_=xB[:, :, 1024:1536].bitcast(f32r)
            )
            nc.sync.dma_start(
                out=xs[:, :, 1536:2048], in_=xB[:, :, 1536:2048].bitcast(f32r)
            )

            # exp ACT table preload while DMAs stream
            preld = singles.tile([32, 32], f32, tag="preld")
            nc.vector.memset(preld, 0.0)
            nc.scalar.activation(out=preld, in_=preld, func=Exp, scale=1.0)

            woT = singles.tile([128, HID], bf16, tag="woT")
            nc.vector.tensor_copy(woT, woT_f)

            # identities + causal mask, generated on device (gpsimd)
            # two stacked 64x64 identities so both head slices (base
            # partition 0 and 64) can use the matching partition range
            identf = singles.tile([128, 64], f32, tag="identf")
            nc.gpsimd.memset(identf, 1.0)
            nc.gpsimd.affine_select(
                out=identf[0:64, :], in_=identf[0:64, :], compare_op=ALU.is_equal,
                fill=0.0, base=0, pattern=[[-1, 64]], channel_multiplier=1,
            )
            nc.gpsimd.affine_select(
                out=identf[64:128, :], in_=identf[64:128, :], compare_op=ALU.is_equal,
                fill=0.0, base=0, pattern=[[-1, 64]], channel_multiplier=1,
            )
            ident64 = singles.tile([128, 64], f32r, tag="ident64")
            nc.vector.tensor_copy(ident64, identf)
            identb = singles.tile([128, 128], bf16, tag="identb")
            nc.gpsimd.memset(identb, 1.0)
            nc.gpsimd.affine_select(
                out=identb, in_=identb, compare_op=ALU.is_equal,
                fill=0.0, base=0, pattern=[[-1, 128]], channel_multiplier=1,
            )
            # trib[k, q] = 1 if k <= q else 0   (iota = q - k >= 0)
            trib = singles.tile([128, 128], bf16, tag="trib")
            nc.gpsimd.memset(trib, 1.0)
            nc.gpsimd.affine_select(
                out=trib, in_=trib, compare_op=ALU.is_ge,
                fill=0.0, base=0, pattern=[[1, 128]], channel_multiplier=-1,
            )

            qT = singles.tile([128, S], f32r, tag="qT")
            v_sb = [
                singles.tile([128, 16, 65], bf16, tag=f"v{h}", name=f"v{h}")
                for h in range(2)
            ]
            for h in range(2):
                nc.gpsimd.memset(v_sb[h][:, :, 64:65], 1.0)

            # state shared across the emission helpers
            et_map = {}    # (h, kc) -> (et_tile, col_of_qc0, first_valid_qc)
            ao_tiles = {}  # (h, qc_local) -> ao tile (bf16 [128, 64])
            av_cur = {}    # h -> av psum tile
            aot_ps = {}    # qb -> psum tile [64, 8, 128] bf16
            aot_sb = {}    # qb -> sbuf tile [128, 4, 128] bf16

            # ---------------- emission helpers ----------------
            def qproj(qb, halves=False):
                s0 = qb * SB
                qp = ppp.tile([128, SB], f32, tag="pp", name=f"qp{qb}")
                parts = [(0, 256), (256, 512)] if halves else [(0, SB)]
                for c0, c1 in parts:
                    for i in range(4):
                        nc.tensor.matmul(
                            qp[:, c0:c1], lhsT=wq[:, i, :],
                            rhs=xs[:, i, s0 + c0 : s0 + c1],
                            start=(i == 0), stop=(i == 3),
                        )
                nc.vector.tensor_scalar_add(qT[:, s0 : s0 + SB], qp, wqb)

            def vprep(h, qb):
                hp = 64 * h
                vt = ppp.tile([128, 4, 64], f32r, tag="pp", name=f"vt{h}_{qb}")
                for j in range(4):
                    t0 = 128 * (4 * qb + j)
                    nc.tensor.transpose(
                        vt[:, j, :], qT[hp : hp + 64, t0 : t0 + 128],
                        ident64[hp : hp + 64, :],
                    )
                nc.vector.tensor_copy(v_sb[h][:, 4 * qb : 4 * qb + 4, 0:64], vt)

            def qk_group(h, qb, chunks, expw, masks):
                """chunks: [(kc, coff, qoff, N)]; masks: [(col, )] tri blocks."""
                hp = 64 * h
                s0 = qb * SB
                qk = qkp.tile([128, 1024], f32, tag="qk", name="qk")
                et = etp.tile([128, 1024], bf16, tag="et", name="et")
                for kc, coff, qoff, n in chunks:
                    t0 = 128 * kc
                    nc.tensor.matmul(
                        qk[:, coff : coff + n],
                        lhsT=qT[hp : hp + 64, t0 : t0 + 128],
                        rhs=qT[hp : hp + 64, s0 + qoff : s0 + qoff + n],
                        start=True, stop=True,
                    )
                    # col of qc_local=fq block, first valid qc_local
                    fq = qoff // 128
                    et_map[(h, kc)] = (et, coff - 128 * fq, fq)
                nc.scalar.activation(
                    out=et[:, 0:expw], in_=qk[:, 0:expw], func=Exp, scale=SCALE
                )
                for mc in masks:
                    nc.vector.tensor_mul(
                        et[:, mc : mc + 128], et[:, mc : mc + 128], trib
                    )

            def unit_groups(h, qb):
                """Emit-list of thunks for the QK/exp phase of unit (h, qb)."""
                k0 = 4 * qb
                gs = []
                for ke in range(0, k0, 2):  # off-diagonal pairs, full width
                    gs.append(
                        lambda ke=ke: qk_group(
                            h, qb,
                            [(ke, 0, 0, 512), (ke + 1, 512, 0, 512)],
                            1024, [],
                        )
                    )
                # diagonal pack A: kc0 (N=512) + kc1 (N=384)
                gs.append(
                    lambda: qk_group(
                        h, qb,
                        [(k0, 0, 0, 512), (k0 + 1, 512, 128, 384)],
                        896, [0, 512],
                    )
                )
                # diagonal pack B: kc2 (N=256) + kc3 (N=256, 128 padded)
                gs.append(
                    lambda: qk_group(
                        h, qb,
                        [(k0 + 2, 0, 256, 256), (k0 + 3, 256, 256, 256)],
                        512, [0, 384],
                    )
                )
                return gs

            def av_item(h, qb, qc_local):
                qc = 4 * qb + qc_local
                if qc_local == 0:
                    av_cur[h] = avp.tile([128, 4, 65], f32, tag="av", name=f"av{h}{qb}")
                av = av_cur[h]
                for kc in range(qc + 1):
                    et, c0, fq = et_map[(h, kc)]
                    nc.tensor.matmul(
                        av[:, qc_local, :],
                        lhsT=et[:, c0 + 128 * qc_local : c0 + 128 * qc_local + 128],
                        rhs=v_sb[h][:, kc, :],
                        start=(kc == 0), stop=(kc == qc),
                    )

            def norm_item(h, qb):
                av = av_cur[h]
                rec = recp.tile([128, 4, 1], f32, tag="rec", name="rec")
                nc.vector.reciprocal(rec, av[:, :, 64:65])
                for qc_local in range(4):
                    ao = aop.tile([128, 64], bf16, tag="ao", name="ao")
                    nc.vector.tensor_scalar_mul(
                        ao, av[:, qc_local, 0:64], rec[:, qc_local, :]
                    )
                    ao_tiles[(h, qc_local)] = ao

            def t_item(h, qb):
                if h == 0:
                    aot_ps[qb] = aotp.tile([64, 8, 128], bf16, tag="aot", name="aot")
                ps = aot_ps[qb]
                for qc_local in range(4):
                    nc.tensor.transpose(
                        ps[:, 4 * h + qc_local, :],
                        ao_tiles[(h, qc_local)], identb,
                    )

            def aot_copy(qb):
                sb = aotsb.tile([128, 4, 128], bf16, tag="aotsb", name="aotsb")
                ps = aot_ps[qb]
                nc.vector.tensor_copy(sb[0:64, :, :], ps[:, 0:4, :])
                nc.vector.tensor_copy(sb[64:128, :, :], ps[:, 4:8, :])
                aot_sb[qb] = sb

            def w_item(qb, qc_local, copy_eng):
                wp = ppp.tile([128, SB], f32, tag="pp", name=f"wp{qb}{qc_local}")
                nc.tensor.matmul(
                    wp, lhsT=aot_sb[qb][:, qc_local, :], rhs=woT,
                    start=True, stop=True,
                )
                ob = obp.tile([128, SB], bf16, tag="ob", name="ob")
                copy_eng.tensor_copy(ob, wp)
                r0 = 512 * qb + 128 * qc_local
                nc.sync.dma_start(out=out_part[r0 : r0 + 128, :], in_=ob)

            def w_items(qb):
                out = []
                for qc_local in range(4):
                    eng = nc.vector if qc_local % 2 == 0 else nc.gpsimd
                    out.append(lambda q=qb, c=qc_local, e=eng: w_item(q, c, e))
                return out

            def av_norm_t(h, qb):
                return [lambda c=c: av_item(h, qb, c) for c in range(4)] + [
                    lambda: norm_item(h, qb),
                    lambda: t_item(h, qb),
                ]

            def emit_unit(h, qb, hk):
                gs = unit_groups(h, qb)
                hk = list(hk)
                for g in gs:
                    g()
                    if hk:
                        hk.pop(0)()
                for item in hk:
                    item()

            # ---------------- main schedule ----------------
            qproj(0, halves=True)
            emit_unit(0, 0, [lambda: vprep(0, 0), lambda: vprep(1, 0)])
            emit_unit(1, 0, av_norm_t(0, 0))
            for qb in range(1, 4):
                qproj(qb)
                emit_unit(
                    0, qb,
                    av_norm_t(1, qb - 1)
                    + [lambda q=qb - 1: aot_copy(q)]
                    + [lambda q=qb: vprep(0, q), lambda q=qb: vprep(1, q)],
                )
                emit_unit(1, qb, (w_items(qb - 1) if qb >= 2 else []) + av_norm_t(0, qb))
            # tail
            for item in w_items(2):
                item()
            for item in av_norm_t(1, 3):
                item()
            aot_copy(3)
            for item in w_items(3):
                item()

    nc.finalize()
    return nc


_NC_CACHE = None


def _get_nc():
    global _NC_CACHE
    if _NC_CACHE is None:
        _NC_CACHE = build_nc()
    return _NC_CACHE


def make_in_maps(x, Wq_w, Wq_b, Wo_w):
    x = np.asarray(x, dtype=np.float32)
    Wq_w = np.asarray(Wq_w, dtype=np.float32)
    Wq_b = np.asarray(Wq_b, dtype=np.float32)
    Wo_w = np.asarray(Wo_w, dtype=np.float32)
    in_maps = []
    for c in range(N_CORES):
        b, hp = divmod(c, 4)
        dq = slice(128 * hp, 128 * (hp + 1))
        xB = np.ascontiguousarray(
            x[b].T.reshape(4, 128, S).transpose(1, 0, 2)
        )
        WqB = np.ascontiguousarray(
            Wq_w[dq, :].T.reshape(4, 128, 128).transpose(1, 0, 2)
        )
        in_maps.append({
            "xB": xB,
            "WqB": WqB,
            "Wqb": np.ascontiguousarray(Wq_b[dq].reshape(128, 1)),
            "WoT": np.ascontiguousarray(Wo_w[:, dq].T),
        })
    return in_maps


def kernel(x, mask, Wq_w, Wq_b, Wo_w, Wo_b, **_):
    nc = _get_nc()
    in_maps = make_in_maps(x, Wq_w, Wq_b, Wo_w)
    res = run_bass_kernel_spmd(nc, in_maps, core_ids=list(range(N_CORES)))
    Wo_b = np.asarray(Wo_b, dtype=np.float32)
    out = np.empty((B, S, HID), dtype=np.float32)
    for b in range(B):
        acc = np.asarray(res.results[4 * b]["out_part"], dtype=np.float32)
        for c in range(4 * b + 1, 4 * b + 4):
            acc = acc + np.asarray(res.results[c]["out_part"], dtype=np.float32)
        out[b] = acc + Wo_b[None, :]
    return out


# revision 6
# speedup vs baseline: 1.3160x; 1.0717x over previous
"""Trainium2 Bass kernel for nn_Attention_36404142801494.

Fused causal self-attention (q=k=v=Wq(x)) + output projection, sharded over
8 NeuronCores: data-parallel on batch (B=2 -> 2 groups of 4 cores), tensor-
parallel on heads (8 heads -> 2 heads/core, dq = 128 hidden dims/core) with a
column-split Wq and a row-split Wo. Each core returns a partial [S, HID]
output (bf16); the host sums the 4 partials per batch and adds the Wo bias.

Per-core structure (keyed to the TimelineSim cost model, where a matmul costs
out_free_size x cycles_per_row and LDWEIGHTS is free):
  - qT [d=128, s] f32r via Wq matmuls (moving = x blocks, N=512/256).
  - QK emits scoresT tiles [k=128, q<=512] per key-chunk, causally trimmed;
    f32r keeps 1 cyc/row at N>=256 (diagonal chunks padded to N=256 min).
  - exp on ACT into bf16 et tiles; diagonal 128x128 blocks masked by a
    lower-triangular bf16 mask multiply on DVE (mask generated on device).
  - AV runs TRANSPOSED: stationary = et 128x128 block, moving = V chunk
    [k=128, 65] bf16 (col 64 = ones) -> av psum [q=128, 65], N=65/matmul.
    Col 64 accumulates the softmax denominator for free; normalization is a
    per-partition reciprocal + tensor_scalar multiply (no DRAM bounce).
  - ao [q, d] bf16 is PE-transposed (bf16 identity) to aoT [d, q]; both
    heads stack to [128, q] so Wo is one K=128 matmul per 128-query chunk
    (moving = WoT [128 d, 512 c] bf16, N=512).
  - Output partials DMA out as bf16 [2048, 512].

Schedule: units (h, qb) are ordered so qproj of the next block lands where
the ACT engine still has exp backlog, and the final unit drains through a
per-query-chunk tail chain (AV -> recip/mul -> transpose -> copy -> Wo ->
DMA) that overlaps the last exp groups.

Everything is hardcoded for B=2, S=2048, HID=512, NH=8, HD=64.
"""

import sys

sys.path.insert(0, "/opt/trn_rl_repo")

import numpy as np

import concourse.bass as bass
import concourse.bacc as bacc
import concourse.tile as tile
import concourse.mybir as mybir
from concourse.bass_utils import run_bass_kernel_spmd

f32 = mybir.dt.float32
f32r = mybir.dt.float32r
bf16 = mybir.dt.bfloat16

B, S, HID = 2, 2048, 512
NH, HD = 8, 64
N_CORES = 8
SB = 512            # query-block width
NSB = S // SB       # 4 query blocks
SCALE = 1.0 / np.sqrt(HD)

Exp = mybir.ActivationFunctionType.Exp
ALU = mybir.AluOpType


def build_nc():
    nc = bacc.Bacc(None, target_bir_lowering=False)

    # host pre-arranged layouts (see make_in_maps):
    #   xB[p, i, s]  = x[b].T[128*i + p, s]
    #   WqB[p, i, c] = Wq_w[dq, :].T[128*i + p, c]
    #   WoT[d, c]    = Wo_w[:, dq].T[d, c]
    xB = nc.dram_tensor("xB", [128, 4, S], f32, kind="ExternalInput")
    WqB = nc.dram_tensor("WqB", [128, 4, 128], f32, kind="ExternalInput")
    Wqb = nc.dram_tensor("Wqb", [128, 1], f32, kind="ExternalInput")
    WoT = nc.dram_tensor("WoT", [128, HID], f32, kind="ExternalInput")
    out_part = nc.dram_tensor("out_part", [S, HID], bf16, kind="ExternalOutput")

    with tile.TileContext(nc) as tc:
        with (
            tc.tile_pool(name="singles", bufs=1) as singles,
            tc.tile_pool(name="etp", bufs=12) as etp,
            tc.tile_pool(name="aop", bufs=6) as aop,
            tc.tile_pool(name="recp", bufs=2) as recp,
            tc.tile_pool(name="aotsb", bufs=2) as aotsb,
            tc.tile_pool(name="obp", bufs=4) as obp,
            tc.tile_pool(name="qkp", bufs=2, space="PSUM") as qkp,
            tc.tile_pool(name="ppp", bufs=2, space="PSUM") as ppp,
            tc.tile_pool(name="avp", bufs=1, space="PSUM") as avp,
            tc.tile_pool(name="aotp", bufs=1, space="PSUM") as aotp,
        ):
            # ---------------- prologue: DMAs + constants ----------------
            wq = singles.tile([128, 4, 128], f32r, tag="wq")
            nc.sync.dma_start(out=wq, in_=WqB[:, :, :].bitcast(f32r))

            xs = singles.tile([128, 4, S], f32r, tag="xs")
            # qb0 in two halves so qproj(0) can start sooner
            nc.sync.dma_start(
                out=xs[:, :, 0:256], in_=xB[:, :, 0:256].bitcast(f32r)
            )
            nc.sync.dma_start(
                out=xs[:, :, 256:512], in_=xB[:, :, 256:512].bitcast(f32r)
            )
            wqb = singles.tile([128, 1], f32, tag="wqb")
            nc.sync.dma_start(out=wqb, in_=Wqb[:, :])
            nc.sync.dma_start(
                out=xs[:, :, 512:1024], in_=xB[:, :, 512:1024].bitcast(f32r)
            )
            woT_f = singles.tile([128, HID], f32, tag="woT_f")
            nc.sync.dma_start(out=woT_f, in_=WoT[:, :])
            nc.sync.dma_start(
                out=xs[:, :, 1024:1536], in_=xB[:, :, 1024:1536].bitcast(f32r)
            )
            nc.sync.dma_start(
                out=xs[:, :, 1536:2048], in_=xB[:, :, 1536:2048].bitcast(f32r)
            )

            # exp ACT table preload while DMAs stream
            preld = singles.tile([32, 32], f32, tag="preld")
            nc.vector.memset(preld, 0.0)
            nc.scalar.activation(out=preld, in_=preld, func=Exp, scale=1.0)

            woT = singles.tile([128, HID], bf16, tag="woT")
            nc.vector.tensor_copy(woT, woT_f)

            # two stacked 64x64 identities so both head slices (base
            # partition 0 and 64) can use the matching partition range
            identf = singles.tile([128, 64], f32, tag="identf")
            nc.gpsimd.memset(identf, 1.0)
            for p0 in (0, 64):
                nc.gpsimd.affine_select(
                    out=identf[p0 : p0 + 64, :], in_=identf[p0 : p0 + 64, :],
                    compare_op=ALU.is_equal,
                    fill=0.0, base=0, pattern=[[-1, 64]], channel_multiplier=1,
                )
            ident64 = singles.tile([128, 64], f32r, tag="ident64")
            nc.vector.tensor_copy(ident64, identf)
            identb = singles.tile([128, 128], bf16, tag="identb")
            nc.gpsimd.memset(identb, 1.0)
            nc.gpsimd.affine_select(
                out=identb, in_=identb, compare_op=ALU.is_equal,
                fill=0.0, base=0, pattern=[[-1, 128]], channel_multiplier=1,
            )
            # trib[k, q] = 1 if k <= q else 0   (iota = q - k >= 0)
            trib = singles.tile([128, 128], bf16, tag="trib")
            nc.gpsimd.memset(trib, 1.0)
            nc.gpsimd.affine_select(
                out=trib, in_=trib, compare_op=ALU.is_ge,
                fill=0.0, base=0, pattern=[[1, 128]], channel_multiplier=-1,
            )

            qT = singles.tile([128, S], f32r, tag="qT")
            v_sb = [
                singles.tile([128, 16, 65], bf16, tag=f"v{h}", name=f"v{h}")
                for h in range(2)
            ]
            for h in range(2):
                nc.gpsimd.memset(v_sb[h][:, :, 64:65], 1.0)

            # state shared across the emission helpers
            et_map = {}    # (h, kc) -> (et_tile, col_of_qc0)
            ao_tiles = {}  # (h, qc_local) -> ao tile (bf16 [128, 64])
            av_cur = {}    # h -> av psum tile
            aot_ps = {}    # qb -> psum tile [64, 8, 128] bf16
            aot_sb = {}    # qb -> sbuf tile [128, 4, 128] bf16

            # ---------------- emission helpers ----------------
            def qproj(qb, halves=False):
                s0 = qb * SB
                qp = ppp.tile([128, SB], f32, tag="pp", name=f"qp{qb}")
                if halves:
                    for c0, c1 in ((0, 256), (256, 512)):
                        for i in range(4):
                            nc.tensor.matmul(
                                qp[:, c0:c1], lhsT=wq[:, i, :],
                                rhs=xs[:, i, s0 + c0 : s0 + c1],
                                start=(i == 0), stop=(i == 3),
                            )
                        nc.vector.tensor_scalar_add(
                            qT[:, s0 + c0 : s0 + c1], qp[:, c0:c1], wqb
                        )
                else:
                    for i in range(4):
                        nc.tensor.matmul(
                            qp, lhsT=wq[:, i, :], rhs=xs[:, i, s0 : s0 + SB],
                            start=(i == 0), stop=(i == 3),
                        )
                    nc.vector.tensor_scalar_add(qT[:, s0 : s0 + SB], qp, wqb)

            def vprep(h, qb):
                hp = 64 * h
                vt = ppp.tile([128, 4, 64], f32r, tag="pp", name=f"vt{h}_{qb}")
                for j in range(4):
                    t0 = 128 * (4 * qb + j)
                    nc.tensor.transpose(
                        vt[:, j, :], qT[hp : hp + 64, t0 : t0 + 128],
                        ident64[hp : hp + 64, :],
                    )
                nc.vector.tensor_copy(v_sb[h][:, 4 * qb : 4 * qb + 4, 0:64], vt)

            def qk_group(h, qb, chunks, expw, masks):
                """chunks: [(kc, coff, qoff, N)]; masks: [col] of tri blocks."""
                hp = 64 * h
                s0 = qb * SB
                qk = qkp.tile([128, 1024], f32, tag="qk", name="qk")
                et = etp.tile([128, 1024], bf16, tag="et", name="et")
                for kc, coff, qoff, n in chunks:
                    t0 = 128 * kc
                    nc.tensor.matmul(
                        qk[:, coff : coff + n],
                        lhsT=qT[hp : hp + 64, t0 : t0 + 128],
                        rhs=qT[hp : hp + 64, s0 + qoff : s0 + qoff + n],
                        start=True, stop=True,
                    )
                    # col where qc_local 0's block would sit
                    et_map[(h, kc)] = (et, coff - 128 * (qoff // 128))
                nc.scalar.activation(
                    out=et[:, 0:expw], in_=qk[:, 0:expw], func=Exp, scale=SCALE
                )
                for mc in masks:
                    nc.vector.tensor_mul(
                        et[:, mc : mc + 128], et[:, mc : mc + 128], trib
                    )

            def unit_groups(h, qb, split_first=False):
                """Emit-list of thunks for the QK/exp phase of unit (h, qb)."""
                k0 = 4 * qb
                gs = []
                for ke in range(0, k0, 2):  # off-diagonal pairs, full width
                    gs.append(
                        lambda ke=ke: qk_group(
                            h, qb,
                            [(ke, 0, 0, 512), (ke + 1, 512, 0, 512)],
                            1024, [],
                        )
                    )
                # diagonal pack A: kc0 (N=512) + kc1 (N=384)
                ch0 = (
                    [(k0, 0, 0, 256), (k0, 256, 256, 256)]
                    if split_first else [(k0, 0, 0, 512)]
                )
                gs.append(
                    lambda: qk_group(
                        h, qb, ch0 + [(k0 + 1, 512, 128, 384)],
                        896, [0, 512],
                    )
                )
                # diagonal pack B: kc2 (N=256) + kc3 (N=256, 128 padded)
                gs.append(
                    lambda: qk_group(
                        h, qb,
                        [(k0 + 2, 0, 256, 256), (k0 + 3, 256, 256, 256)],
                        512, [0, 384],
                    )
                )
                return gs

            def av_item(h, qb, qc_local):
                qc = 4 * qb + qc_local
                if qc_local == 0:
                    av_cur[h] = avp.tile(
                        [128, 4, 65], f32, tag="av", name=f"av{h}{qb}"
                    )
                av = av_cur[h]
                for kc in range(qc + 1):
                    et, c0 = et_map[(h, kc)]
                    nc.tensor.matmul(
                        av[:, qc_local, :],
                        lhsT=et[:, c0 + 128 * qc_local : c0 + 128 * qc_local + 128],
                        rhs=v_sb[h][:, kc, :],
                        start=(kc == 0), stop=(kc == qc),
                    )

            def norm_item(h, qb):
                av = av_cur[h]
                rec = recp.tile([128, 4, 1], f32, tag="rec", name="rec")
                nc.vector.reciprocal(rec, av[:, :, 64:65])
                for qc_local in range(4):
                    ao = aop.tile([128, 64], bf16, tag="ao", name="ao")
                    nc.vector.tensor_scalar_mul(
                        ao, av[:, qc_local, 0:64], rec[:, qc_local, :]
                    )
                    ao_tiles[(h, qc_local)] = ao

            def t_item(h, qb):
                if h == 0:
                    aot_ps[qb] = aotp.tile(
                        [64, 8, 128], bf16, tag="aot", name="aot"
                    )
                ps = aot_ps[qb]
                for qc_local in range(4):
                    nc.tensor.transpose(
                        ps[:, 4 * h + qc_local, :],
                        ao_tiles[(h, qc_local)], identb,
                    )

            def aot_copy(qb, h=None):
                if qb not in aot_sb:
                    aot_sb[qb] = aotsb.tile(
                        [128, 4, 128], bf16, tag="aotsb", name="aotsb"
                    )
                sb = aot_sb[qb]
                ps = aot_ps[qb]
                if h in (None, 0):
                    nc.vector.tensor_copy(sb[0:64, :, :], ps[:, 0:4, :])
                if h in (None, 1):
                    nc.vector.tensor_copy(sb[64:128, :, :], ps[:, 4:8, :])

            def w_item(qb, qc_local, copy_eng):
                wp = ppp.tile([128, SB], f32, tag="pp", name=f"wp{qb}{qc_local}")
                nc.tensor.matmul(
                    wp, lhsT=aot_sb[qb][:, qc_local, :], rhs=woT,
                    start=True, stop=True,
                )
                ob = obp.tile([128, SB], bf16, tag="ob", name="ob")
                copy_eng.tensor_copy(ob, wp)
                r0 = 512 * qb + 128 * qc_local
                nc.sync.dma_start(out=out_part[r0 : r0 + 128, :], in_=ob)

            def w_items(qb):
                out = []
                for qc_local in range(4):
                    eng = nc.vector if qc_local % 2 == 0 else nc.gpsimd
                    out.append(lambda q=qb, c=qc_local, e=eng: w_item(q, c, e))
                return out

            def av_norm_t(h, qb):
                return [lambda c=c: av_item(h, qb, c) for c in range(4)] + [
                    lambda: norm_item(h, qb),
                    lambda: t_item(h, qb),
                ]

            def tail_qc(qc_local, ob_on_act):
                """Per-query-chunk drain chain for the last unit (h=1, qb=3)."""
                h, qb = 1, 3
                av_item(h, qb, qc_local)
                av = av_cur[h]
                rec = recp.tile([128, 1, 1], f32, tag="rec", name="rect")
                nc.vector.reciprocal(rec, av[:, qc_local, 64:65])
                ao = aop.tile([128, 64], bf16, tag="ao", name="ao")
                nc.vector.tensor_scalar_mul(
                    ao, av[:, qc_local, 0:64], rec[:, 0, :]
                )
                ps = aot_ps[qb]
                nc.tensor.transpose(ps[:, 4 + qc_local, :], ao, identb)
                sb = aot_sb[qb]
                nc.vector.tensor_copy(
                    sb[64:128, qc_local, :], ps[:, 4 + qc_local, :]
                )
                wp = ppp.tile([128, SB], f32, tag="pp", name=f"wpt{qc_local}")
                nc.tensor.matmul(
                    wp, lhsT=sb[:, qc_local, :], rhs=woT, start=True, stop=True
                )
                ob = obp.tile([128, SB], bf16, tag="ob", name="ob")
                if ob_on_act:
                    nc.scalar.copy(ob, wp)
                else:
                    nc.gpsimd.tensor_copy(ob, wp)
                r0 = 512 * qb + 128 * qc_local
                nc.sync.dma_start(out=out_part[r0 : r0 + 128, :], in_=ob)

            def emit_unit(h, qb, hk, split_first=False):
                gs = unit_groups(h, qb, split_first=split_first)
                hk = list(hk)
                for g in gs:
                    g()
                    if hk:
                        hk.pop(0)()
                for item in hk:
                    item()

            # ---------------- main schedule ----------------
            qproj(0, halves=True)
            emit_unit(0, 0, [lambda: vprep(0, 0), lambda: vprep(1, 0)],
                      split_first=True)
            qproj(1)
            emit_unit(1, 0, [lambda: vprep(0, 1), lambda: vprep(1, 1)]
                      + av_norm_t(0, 0), split_first=True)
            emit_unit(0, 1, av_norm_t(1, 0) + [lambda: aot_copy(0)])
            qproj(2)
            emit_unit(1, 1, [lambda: vprep(0, 2), lambda: vprep(1, 2)]
                      + w_items(0) + av_norm_t(0, 1))
            emit_unit(0, 2, av_norm_t(1, 1) + [lambda: aot_copy(1)])
            qproj(3)
            emit_unit(1, 2, [lambda: vprep(0, 3), lambda: vprep(1, 3)]
                      + w_items(1) + av_norm_t(0, 2))
            emit_unit(0, 3, av_norm_t(1, 2) + [lambda: aot_copy(2)])
            # last unit: front-load its own AV(0,3)/T(0,3) prep, then drain
            # through per-qc tail chains that overlap the final exp groups
            emit_unit(
                1, 3,
                [lambda c=c: av_item(0, 3, c) for c in range(4)]
                + [lambda: norm_item(0, 3), lambda: t_item(0, 3),
                   lambda: aot_copy(3, h=0)]
                + w_items(2),
            )
            tail_qc(0, ob_on_act=False)
            tail_qc(1, ob_on_act=False)
            tail_qc(2, ob_on_act=True)
            tail_qc(3, ob_on_act=True)

    nc.finalize()
    return nc


_NC_CACHE = None


def _get_nc():
    global _NC_CACHE
    if _NC_CACHE is None:
        _NC_CACHE = build_nc()
    return _NC_CACHE


def make_in_maps(x, Wq_w, Wq_b, Wo_w):
    x = np.asarray(x, dtype=np.float32)
    Wq_w = np.asarray(Wq_w, dtype=np.float32)
    Wq_b = np.asarray(Wq_b, dtype=np.float32)
    Wo_w = np.asarray(Wo_w, dtype=np.float32)
    in_maps = []
    for c in range(N_CORES):
        b, hp = divmod(c, 4)
        dq = slice(128 * hp, 128 * (hp + 1))
        xB = np.ascontiguousarray(
            x[b].T.reshape(4, 128, S).transpose(1, 0, 2)
        )
        WqB = np.ascontiguousarray(
            Wq_w[dq, :].T.reshape(4, 128, 128).transpose(1, 0, 2)
        )
        in_maps.append({
            "xB": xB,
            "WqB": WqB,
            "Wqb": np.ascontiguousarray(Wq_b[dq].reshape(128, 1)),
            "WoT": np.ascontiguousarray(Wo_w[:, dq].T),
        })
    return in_maps


def kernel(x, mask, Wq_w, Wq_b, Wo_w, Wo_b, **_):
    nc = _get_nc()
    in_maps = make_in_maps(x, Wq_w, Wq_b, Wo_w)
    res = run_bass_kernel_spmd(nc, in_maps, core_ids=list(range(N_CORES)))
    Wo_b = np.asarray(Wo_b, dtype=np.float32)
    out = np.empty((B, S, HID), dtype=np.float32)
    for b in range(B):
        acc = np.asarray(res.results[4 * b]["out_part"], dtype=np.float32)
        for c in range(4 * b + 1, 4 * b + 4):
            acc = acc + np.asarray(res.results[c]["out_part"], dtype=np.float32)
        out[b] = acc + Wo_b[None, :]
    return out
